# revision 27
# baseline (speedup 1.0000x reference)
"""Causal self-attention (GPT-style) Trainium2 Bass kernel.

Problem: x[2,4096,768] -> qkv = x@W_attn+b_attn -> 12-head causal attention
-> out @ W_proj + b_proj.   B=2, T=4096, C=768, H=12, Dh=64.

Sharding: (batch, head) parallel over 8 cores. Core c handles batch c//4 and
heads 3*(c%4) .. 3*(c%4)+2.  Each core computes qkv for its 3 heads, causal
attention, and a partial output projection (rows of W_proj for its heads).
Host sums the 4 partials per batch (+ b_proj once) and transposes.

Design (v3, software-pipelined, bf16 scores):
- Scores are computed transposed (S^T[k,q]) so the exp output P^T is directly
  the lhsT of a natural-layout PV matmul: out att[q128, dv65] with full
  128-wide output partitions (2x fewer PE cycles than the att^T[65, q] form).
- Z (softmax denom) comes free from a ones-column appended to V; the PSUM
  att tile is normalized by a per-partition reciprocal during the PSUM->SBUF
  copy (tensor_scalar), eliminating any DRAM broadcast round-trip.
- att^T for the output projection is produced by XBAR DMA transposes
  (SBUF->SBUF) instead of PE transposes + copies.
- Everything is emitted through a software pipeline: while the Activation
  engine (the bottleneck: ~26M exp elements) chews on chunk n's score tiles,
  the PE runs chunk n's PV accumulation, chunk n+1's QKV projection and the
  per-qsub output projection as 'filler' work between score matmuls, paced
  by an adaptive budget (Act group time minus the score matmul time).
- Causal structure: only lower-triangle k-tiles are computed; diagonal tiles
  have restricted q-ranges in the score matmul AND the exp, plus a
  triangular band mask multiplied into P^T.
- PSUM budget (8 banks): score ring 2x[128,2,512]f32 (4 banks) + PV
  accumulators 2x[128,3,65]f32 (2) + shared proj/out-proj ring 2x1 bank.
  The 3 PV head-slices (and the 3 out-proj m-slices) share one bank: only
  the first matmul of a bank uses start=True; later slice-groups rely on
  the pending-zero region semantics.
"""

import math
import sys
from collections import deque

sys.path.insert(0, "/opt/trn_rl_repo")

import numpy as np
import ml_dtypes

B, T, C = 2, 4096, 768
NH, DH = 12, 64
HPC = 3          # heads per core
NCORES = 8
NQ = T // 512    # q chunks (8)
NKT = T // 128   # k tiles (32)

BF16 = ml_dtypes.bfloat16

_PROG = None


def _build_program():
    import concourse.bass as bass
    import concourse.mybir as mybir
    import concourse.tile as tile
    from concourse import bacc

    f32 = mybir.dt.float32
    bf16 = mybir.dt.bfloat16
    Exp = mybir.ActivationFunctionType.Exp
    mult = mybir.AluOpType.mult
    add = mybir.AluOpType.add

    nc = bacc.Bacc("TRN2", target_bir_lowering=False)

    # ---- I/O ----
    xt_d = nc.dram_tensor("xt", [C, T], bf16, kind="ExternalInput")     # x[b].T
    # wqk columns: [Q0|Q1|K0|K1|Q2|K2] (64 each)
    wqk_d = nc.dram_tensor("wqk", [C, 384], bf16, kind="ExternalInput")
    qkb_d = nc.dram_tensor("qkb", [128, 3], f32, kind="ExternalInput")  # biases
    wv_d = nc.dram_tensor("wv", [C, 195], bf16, kind="ExternalInput")   # [Wv|0]x3
    wvb_d = nc.dram_tensor("wvb", [1, 195], bf16, kind="ExternalInput")  # [b_v|1]x3
    wp_d = nc.dram_tensor("wp", [192, 768], bf16, kind="ExternalInput")
    mask_d = nc.dram_tensor("mask", [128, 128], bf16, kind="ExternalInput")
    eye_d = nc.dram_tensor("eye", [128, 128], bf16, kind="ExternalInput")
    yt_d = nc.dram_tensor("yt", [C, T], bf16, kind="ExternalOutput")    # y[b].T part

    xt_r = xt_d.rearrange("(g p) t -> p g t", p=128)   # [128, 6, T]
    yt_r = yt_d.rearrange("(g p) t -> p g t", p=128)   # [128, 6, T]

    with tile.TileContext(nc) as tc:
        with (
            tc.tile_pool(name="const", bufs=1) as const,
            tc.tile_pool(name="pers", bufs=1) as pers,
            tc.tile_pool(name="xtp", bufs=4) as xtp,
            tc.tile_pool(name="ptp", bufs=50) as ptp,
            tc.tile_pool(name="atn", bufs=4) as atn,
            tc.tile_pool(name="at0", bufs=2) as at0p,
            tc.tile_pool(name="at1", bufs=2) as at1p,
            tc.tile_pool(name="ysp", bufs=2) as ysp,
            tc.tile_pool(name="rzp", bufs=6) as rzp,
            tc.tile_pool(name="scp", bufs=2, space="PSUM") as scp,
            tc.tile_pool(name="pvp", bufs=2, space="PSUM") as pvp,
            tc.tile_pool(name="prj", bufs=2, space="PSUM") as prj,
        ):
            # ---- persistent tensors / weights ----
            qk_sb = pers.tile([128, 3, T], bf16)   # [Q0|Q1], [K0|K1], [Q2|K2]
            kq2_sb = pers.tile([128, T], bf16)     # [K2 | Q2] duplicate
            v_sb = pers.tile([128, NKT, 195], bf16)

            wqk_sb = const.tile([128, 6, 384], bf16)
            qkb_sb = const.tile([128, 3], f32)
            wv_sb = const.tile([128, 6, 195], bf16)
            wvbc_sb = const.tile([128, 195], bf16)   # bias bcast (incl Z ones)
            wp0_sb = const.tile([128, 768], bf16)
            wp1_sb = const.tile([64, 768], bf16)
            mask_sb = const.tile([128, 128], bf16)
            eye_sb = const.tile([128, 128], bf16)

            # ---- pipeline state ----
            xt_tiles = {}
            pt_store = {}      # (n, kt, h) -> AP [128, 512] (P^T, bf16)
            attn_tiles = {}    # n -> (at0 tile, at1 tile)
            pv_tiles = {}      # (n, s) -> psum tile
            ysb_tiles = {}     # n -> ysb tile
            fillers = deque()  # (est_pe_ns, fn)

            def pump(budget):
                # no overshoot: a filler only runs if it fits the remaining
                # budget (head-blocking keeps FIFO dependency order)
                while fillers and fillers[0][0] <= budget + 120:
                    cost, fn = fillers.popleft()
                    fn()
                    budget -= cost

            def load_xt(n, split=False):
                t = xtp.tile([128, 6, 512], bf16, name="xt")
                nsl = slice(n * 512, (n + 1) * 512)
                if split:
                    nc.sync.dma_start(out=t[:, 0:3, :], in_=xt_r[:, 0:3, nsl])
                    nc.sync.dma_start(out=t[:, 3:6, :], in_=xt_r[:, 3:6, nsl])
                else:
                    nc.sync.dma_start(out=t, in_=xt_r[:, :, nsl])
                xt_tiles[n] = t

            prjqk_tiles = {}

            def emit_projqk_a(n, m):
                ps = prj.tile([128, 512], f32, name="prjt")
                prjqk_tiles[(n, m)] = ps
                for k in range(3):
                    nc.tensor.matmul(
                        ps, lhsT=wqk_sb[:, k, m * 128:(m + 1) * 128],
                        rhs=xt_tiles[n][:, k, :],
                        start=(k == 0), stop=False,
                    )

            def emit_projqk_b(n, m):
                nsl = slice(n * 512, (n + 1) * 512)
                ps = prjqk_tiles.pop((n, m))
                for k in range(3, 6):
                    nc.tensor.matmul(
                        ps, lhsT=wqk_sb[:, k, m * 128:(m + 1) * 128],
                        rhs=xt_tiles[n][:, k, :],
                        start=False, stop=(k == 5),
                    )
                nc.vector.tensor_scalar(
                    qk_sb[:, m, nsl], ps, qkb_sb[:, m:m + 1], None, add)
                if m == 2:
                    # duplicate [Q2|K2] -> [K2|Q2] for base-matched h2 matmuls
                    nc.sync.dma_start(out=kq2_sb[0:64, nsl],
                                      in_=qk_sb[64:128, 2, nsl])
                    nc.sync.dma_start(out=kq2_sb[64:128, nsl],
                                      in_=qk_sb[0:64, 2, nsl])

            def emit_projqk(n, m):
                emit_projqk_a(n, m)
                emit_projqk_b(n, m)

            def emit_projv(n, tt):
                kt = 4 * n + tt
                ps = prj.tile([128, 195], f32, name="prjt")
                for k in range(6):
                    nc.tensor.matmul(
                        ps, lhsT=xt_tiles[n][:, k, tt * 128:(tt + 1) * 128],
                        rhs=wv_sb[:, k, :],
                        start=(k == 0), stop=(k == 5),
                    )
                nc.vector.tensor_tensor(v_sb[:, kt, :], ps, wvbc_sb, add)

            def K_ap(h, kt):
                sl = slice(kt * 128, (kt + 1) * 128)
                if h == 0:
                    return qk_sb[0:64, 1, sl]
                if h == 1:
                    return qk_sb[64:128, 1, sl]
                return qk_sb[64:128, 2, sl] if kt % 2 == 0 else kq2_sb[0:64, sl]

            def Q_ap(h, n, kt, j0):
                sl = slice(n * 512 + j0, (n + 1) * 512)
                if h == 0:
                    return qk_sb[0:64, 0, sl]
                if h == 1:
                    return qk_sb[64:128, 0, sl]
                return kq2_sb[64:128, sl] if kt % 2 == 0 else qk_sb[0:64, 2, sl]

            def emit_score_group(n, pair):
                sc = scp.tile([128, 2, 512], f32, name="sc")
                pt = ptp.tile([128, 2, 512], bf16, name="pt")
                j0s = []
                for idx, (h, kt) in enumerate(pair):
                    j = kt - 4 * n
                    j0 = 128 * j if j > 0 else 0
                    j0s.append(j0)
                    nc.tensor.matmul(
                        sc[:, idx, j0:], lhsT=K_ap(h, kt), rhs=Q_ap(h, n, kt, j0),
                        start=True, stop=True,
                    )
                if j0s[0] == j0s[1]:
                    nc.scalar.activation(
                        pt[:, :, j0s[0]:], sc[:, :, j0s[0]:], Exp, scale=0.125
                    )
                else:
                    for idx in range(2):
                        nc.scalar.activation(
                            pt[:, idx, j0s[idx]:], sc[:, idx, j0s[idx]:],
                            Exp, scale=0.125,
                        )
                for idx, (h, kt) in enumerate(pair):
                    j = kt - 4 * n
                    if j >= 0:
                        band = pt[:, idx, 128 * j:128 * (j + 1)]
                        nc.gpsimd.tensor_tensor(band, band, mask_sb, mult)
                    pt_store[(n, kt, h)] = pt[:, idx, :]
                # act-time estimate for adaptive pump
                free = sum(512 - j for j in j0s)
                act = 0.833 * free + (370 if j0s[0] != j0s[1] else 185)
                pe = 0.4167 * free
                return act - pe - 30

            def pv_piece(n, s, kts):
                if kts[0] == 0:
                    # last chunk: all 4 qsubs stream concurrently — s>=2
                    # borrows the (otherwise idle) proj-pool banks
                    pool = prj if (s >= 2 and n == NQ - 1) else pvp
                    pv_tiles[(n, s)] = pool.tile(
                        [128, HPC, 65], f32, name="pvt" if pool is pvp else "prjt")
                tl = pv_tiles[(n, s)]
                last = 4 * n + s
                for kt in kts:
                    for h in range(HPC):
                        nc.tensor.matmul(
                            tl[:, h, :],
                            lhsT=pt_store[(n, kt, h)][:, 128 * s:128 * (s + 1)],
                            rhs=v_sb[:, kt, 65 * h:65 * (h + 1)],
                            start=(kt == 0 and h == 0),
                            stop=(kt == last),
                            skip_group_check=True,
                        )

            def pv_drain(n, s):
                tl = pv_tiles.pop((n, s))
                rz = rzp.tile([128, HPC], f32, name="rz")
                nc.vector.reciprocal(rz, tl[:, :, 64])
                an = atn.tile([128, 256], bf16, name="an")
                Copy = mybir.ActivationFunctionType.Copy
                for h in range(HPC):
                    if n == NQ - 1:
                        nc.scalar.activation(
                            an[:, 64 * h:64 * (h + 1)], tl[:, h, 0:64],
                            Copy, scale=rz[:, h:h + 1],
                        )
                    else:
                        nc.vector.tensor_scalar(
                            an[:, 64 * h:64 * (h + 1)], tl[:, h, 0:64],
                            rz[:, h:h + 1], None, mult,
                        )
                a0, a1 = attn_tiles[n]
                qsl = slice(128 * s, 128 * (s + 1))
                if n == NQ - 1 and s >= 2:
                    # tail latency: PE transpose (+copy) instead of the
                    # ~2.4us XBAR DMA round trip; score psum ring is idle
                    tp = scp.tile([128, 256], bf16, name="sc")
                    nc.tensor.transpose(tp[:, 0:128], an[:, 0:128], eye_sb)
                    nc.tensor.transpose(tp[:, 128:256], an[:, 128:256], eye_sb)
                    nc.vector.tensor_copy(a0[:, qsl], tp[:, 0:128])
                    nc.scalar.copy(out=a1[:, qsl], in_=tp[:, 128:256])
                else:
                    nc.sync.dma_start(out=a0[:, qsl], in_=an[:, 0:128],
                                      transpose=True)
                    nc.sync.dma_start(out=a1[:, qsl], in_=an[:, 128:256],
                                      transpose=True)

            def emit_phase4(n, s, half):
                # half 0: m=0..2, half 1: m=3..5 -> one [128, 3, 128] psum tile
                if n not in ysb_tiles:
                    ysb_tiles[n] = ysp.tile([128, 6, 512], bf16, name="ys")
                ysb = ysb_tiles[n]
                a0, a1 = attn_tiles[n]
                qsl = slice(128 * s, 128 * (s + 1))
                ps = prj.tile([128, 3, 128], f32, name="prjt")
                for mi in range(3):
                    m = 3 * half + mi
                    msl = slice(m * 128, (m + 1) * 128)
                    nc.tensor.matmul(ps[:, mi, :], lhsT=wp0_sb[:, msl],
                                     rhs=a0[:, qsl], start=(mi == 0), stop=False,
                                     skip_group_check=True)
                    nc.tensor.matmul(ps[:, mi, :], lhsT=wp1_sb[:, msl],
                                     rhs=a1[0:64, qsl], start=False, stop=True,
                                     skip_group_check=True)
                if n == NQ - 1 and half == 1:
                    nc.scalar.copy(out=ysb[:, 3:6, qsl], in_=ps)
                else:
                    nc.vector.tensor_copy(
                        ysb[:, 3 * half:3 * half + 3, qsl], ps)

            def emit_yt(n, s=None):
                if s is None:
                    nsl = slice(n * 512, (n + 1) * 512)
                    nc.sync.dma_start(out=yt_r[:, :, nsl], in_=ysb_tiles[n])
                else:
                    nsl = slice(n * 512 + 128 * s, n * 512 + 128 * (s + 1))
                    nc.sync.dma_start(out=yt_r[:, :, nsl],
                                      in_=ysb_tiles[n][:, :, 128 * s:128 * (s + 1)])

            # ---- prologue ----
            # PE p-state warmup: ramp the tensor engine to full clock while
            # the first xt DMA is in flight.
            wrm = const.tile([1, 512], bf16)
            nc.vector.memset(wrm, 0.0)
            for _ in range(5):
                wps = prj.tile([1, 512], f32, name="prjt")
                nc.tensor.matmul(wps, lhsT=wrm[:, 0:1], rhs=wrm,
                                 start=True, stop=True)
            # weight loads ride the idle Activation DGE queue so xt streaming
            # isn't serialized behind them on SP
            wqk_r = wqk_d.rearrange("(g p) f -> p g f", p=128)
            nc.scalar.dma_start(out=wqk_sb[:, :, 0:256], in_=wqk_r[:, :, 0:256])
            nc.scalar.dma_start(out=qkb_sb, in_=qkb_d[:, :])
            nc.scalar.dma_start(out=wqk_sb[:, :, 256:384], in_=wqk_r[:, :, 256:384])
            load_xt(0, split=True)
            nc.scalar.dma_start(
                out=wvbc_sb,
                in_=bass.AP(tensor=wvb_d, offset=0, ap=[[0, 128], [1, 195]]),
            )
            nc.scalar.dma_start(out=wp0_sb, in_=wp_d[0:128, :])
            nc.scalar.dma_start(out=wp1_sb, in_=wp_d[128:192, :])
            nc.scalar.dma_start(out=mask_sb, in_=mask_d[:, :])
            nc.scalar.dma_start(out=eye_sb, in_=eye_d[:, :])
            nc.sync.dma_start(out=wv_sb, in_=wv_d.rearrange("(g p) f -> p g f", p=128))
            load_xt(1)
            load_xt(2)
            for m in (0, 1, 2):
                emit_projqk(0, m)

            proj_pending = {}

            def _proj_item(c, cost, fn):
                proj_pending[c] = proj_pending.get(c, 0) + 1

                def run():
                    proj_pending[c] -= 1
                    fn()
                fillers.append((cost, run))

            def enqueue_proj(c):
                for m in range(3):
                    _proj_item(c, 650, lambda c=c, m=m: emit_projqk_a(c, m))
                    _proj_item(c, 650, lambda c=c, m=m: emit_projqk_b(c, m))
                for tt in range(4):
                    _proj_item(c, 570, lambda c=c, tt=tt: emit_projv(c, tt))

            def force_proj(c):
                # chunk c's projections (and everything queued before them)
                # must be emitted before chunk c's score groups / xt reuse
                while proj_pending.get(c, 0) > 0:
                    cost, fn = fillers.popleft()
                    fn()

            for tt in range(4):
                _proj_item(0, 570, lambda tt=tt: emit_projv(0, tt))
            enqueue_proj(1)

            # ---- main loop over q chunks ----
            for n in range(NQ):
                force_proj(n)
                if n + 3 < NQ:
                    load_xt(n + 3)
                if n + 2 < NQ:
                    enqueue_proj(n + 2)

                attn_tiles[n] = (
                    at0p.tile([128, 512], bf16, name="a0"),
                    at1p.tile([128, 512], bf16, name="a1"),
                )
                # s=0,1 stream from the start; s=2 (resp. 3) opens when s=0
                # (resp. 1) closes and catches up.  Last chunk: all 4 stream.
                if n == NQ - 1:
                    pv_next = {0: 0, 1: 0, 2: 0, 3: 0}
                else:
                    pv_next = {0: 0, 1: 0}
                deferred_p4 = []

                if n == 0:
                    # the kq2 duplicate (h2 operand) arrives by DMA ~2.4us
                    # after the m2 projection copy; hide that behind the
                    # h0/h1 exps by ordering all h2 groups last.
                    grps = []
                    for kp in range(2):
                        kA, kB = 2 * kp, 2 * kp + 1
                        grps += [[(0, kA), (1, kA)], [(0, kB), (1, kB)]]
                    for kp in range(2):
                        kA, kB = 2 * kp, 2 * kp + 1
                        grps.append([(2, kA), (2, kB)])
                    for grp in grps:
                        budget = emit_score_group(n, grp)
                        pump(budget)
                    pair_list = [2 * (n + 1) - 1]
                else:
                    pair_list = range(2 * (n + 1))
                for kp in pair_list:
                    kA, kB = 2 * kp, 2 * kp + 1
                    if n > 0:
                        for grp in ([(0, kA), (1, kA)], [(0, kB), (1, kB)],
                                    [(2, kA), (2, kB)]):
                            budget = emit_score_group(n, grp)
                            pump(budget)

                    for s in (0, 1, 2, 3):
                        if s not in pv_next:
                            continue
                        last = 4 * n + s
                        hi = min(kB, last)
                        kts = list(range(pv_next[s], hi + 1))
                        if not kts:
                            continue
                        pv_next[s] = hi + 1
                        for i in range(0, len(kts), 4):
                            piece = kts[i:i + 4]
                            fillers.append(
                                (len(piece) * HPC * 30,
                                 lambda n=n, s=s, piece=piece: pv_piece(n, s, piece)))
                        if hi == last:
                            del pv_next[s]
                            fillers.append(
                                (80, lambda n=n, s=s: pv_drain(n, s)))
                            p4_items = [
                                (330, lambda n=n, s=s, half=half:
                                 emit_phase4(n, s, half))
                                for half in range(2)
                            ]
                            if n == NQ - 1:
                                # out-proj must wait until s2/s3 vacate the
                                # borrowed prj banks
                                deferred_p4 += p4_items
                            else:
                                fillers.extend(p4_items)
                                if s + 2 <= 3:
                                    pv_next[s + 2] = 0
                            if s == 3:
                                fillers.extend(deferred_p4)
                                if n == NQ - 1:
                                    for sq in range(4):
                                        fillers.append(
                                            (30, lambda n=n, sq=sq: emit_yt(n, sq)))
                                else:
                                    fillers.append(
                                        (30, lambda n=n: emit_yt(n)))

                    if n == NQ - 1 and kp >= 2 * n:
                        # final chunk: Act has nothing after the last groups,
                        # so overlapping the PV tail with them is free.
                        pump(700)

            pump(float("inf"))

    nc.finalize()
    return nc


def _get_program():
    global _PROG
    if _PROG is None:
        _PROG = _build_program()
    return _PROG


def _core_inputs(x, W_attn, b_attn, W_proj, b_proj, core):
    b = core // 4
    h0 = HPC * (core % 4)

    def qcol(h):
        return W_attn[:, (h0 + h) * 64:(h0 + h + 1) * 64]

    def kcol(h):
        return W_attn[:, C + (h0 + h) * 64:C + (h0 + h + 1) * 64]

    def vcol(h):
        return W_attn[:, 2 * C + (h0 + h) * 64:2 * C + (h0 + h + 1) * 64]

    def qb(h):
        return b_attn[(h0 + h) * 64:(h0 + h + 1) * 64]

    def kb(h):
        return b_attn[C + (h0 + h) * 64:C + (h0 + h + 1) * 64]

    def vb(h):
        return b_attn[2 * C + (h0 + h) * 64:2 * C + (h0 + h + 1) * 64]

    xt = np.ascontiguousarray(x[b].T).astype(BF16)
    wqk = np.concatenate(
        [qcol(0), qcol(1), kcol(0), kcol(1), qcol(2), kcol(2)], axis=1
    ).astype(BF16)
    qkb = np.stack(
        [
            np.concatenate([qb(0), qb(1)]),
            np.concatenate([kb(0), kb(1)]),
            np.concatenate([qb(2), kb(2)]),
        ],
        axis=1,
    ).astype(np.float32)
    zcol = np.zeros((C, 1), np.float32)
    wv = np.concatenate(
        [np.concatenate([vcol(h), zcol], axis=1) for h in range(3)], axis=1
    ).astype(BF16)
    one = np.ones((1,), np.float32)
    wvb = np.concatenate(
        [np.concatenate([vb(h), one]) for h in range(3)]
    )[None, :].astype(BF16)
    wp = W_proj[h0 * 64:(h0 + HPC) * 64, :].astype(BF16)
    mask = np.triu(np.ones((128, 128), np.float32)).astype(BF16)
    eye = np.eye(128, dtype=np.float32).astype(BF16)
    return {
        "xt": xt, "wqk": wqk, "qkb": qkb, "wv": wv, "wvb": wvb,
        "wp": wp, "mask": mask, "eye": eye,
    }


def _run(x, W_attn, b_attn, W_proj, b_proj, trace=False):
    from concourse.bass_utils import run_bass_kernel_spmd

    x = np.asarray(x, dtype=np.float32)
    W_attn = np.asarray(W_attn, dtype=np.float32)
    b_attn = np.asarray(b_attn, dtype=np.float32)
    W_proj = np.asarray(W_proj, dtype=np.float32)
    b_proj = np.asarray(b_proj, dtype=np.float32)

    nc = _get_program()
    in_maps = [
        _core_inputs(x, W_attn, b_attn, W_proj, b_proj, c) for c in range(NCORES)
    ]
    res = run_bass_kernel_spmd(
        nc, in_maps, core_ids=list(range(NCORES)), trace=trace
    )
    y = np.zeros((B, T, C), np.float32)
    for c in range(NCORES):
        y[c // 4] += res.results[c]["yt"].T.astype(np.float32)
    y += b_proj[None, None, :]
    return y, res


def kernel(x, W_attn, b_attn, W_proj, b_proj):
    y, _ = _run(x, W_attn, b_attn, W_proj, b_proj)
    return y
